# revision 1
# baseline (speedup 1.0000x reference)
"""HeadUpdator kernel for 8 Trainium2 NeuronCores.

Math: the FFT "assembly" step reduces exactly to
    assemble[b, n, c] = sum_spatial(pred_final[b, n]) * sum_spatial(feat_final[b, c])
because irfft2(rfft2(p) * rfft2(f)) is a circular convolution, and summing a
circular convolution over all output positions factors into the product of the
operand sums.

The spatial sum of each zero-padded depthwise conv output factors as
    sum(conv(x, W)) = sum_k W_k * rect_k(x) + H*W*bias
where rect_k is the sum of x over a rectangle missing up to 5 border rows or
cols.  So the device-side work over the 256 MB `feat` tensor is a pure
streaming per-image total-sum (VectorE free-dim reduces, hidden under the HBM
DMA stream); border corrections are computed on host from thin slices of feat
(10 rows + 10 cols + 4 corners per conv channel).

Device (per core, data-parallel over batch: 2 samples/core):
  - 16 x 2MB feat tiles -> one (128, 4096) -> (128, 1) reduce each.
  - pred: host-upsampled image -> Sigmoid chain on ScalarE -> reduces.
Host: exact bilinear x2 upsample, border/corner corrections, the tiny gated
MLP head (16x64 matmuls), and output assembly.
"""

import numpy as np

BS, CH, H, W = 16, 64, 256, 256
NCORES = 8
BL = BS // NCORES            # local batches per core
IMGS = BL * CH               # images per core
HW = H * W
CORE_FLOATS = IMGS * HW      # 8388608 floats of feat per core
# per-tile free-dim sizes (x128 partitions); tapered tail shortens the
# final DMA-dependent reduce
TILE_FREE = [4096] * 15 + [2048] * 2
TILE_OFS = np.cumsum([0] + TILE_FREE[:-1]).tolist()
TILES = len(TILE_FREE)
assert sum(TILE_FREE) * 128 == CORE_FLOATS
# reduce-engine split: ScalarE (activation accum, ~3.7us/2MB tile) alternates
# with VectorE (tensor_scalar accum, ~4.4us/2MB tile) so the per-tile reduce
# keeps pace with the DMA stream on both engines and the tail shrinks to one
# small reduce
ACT_TILES = [t for t in range(TILES) if t % 3 == 2]
VEC_TILES = [t for t in range(TILES) if t % 3 != 2]
LN_EPS = 1e-5

_NC_CACHE = {}
TRACE = False          # test harness sets True to collect an NTFF profile
LAST_RESULTS = None    # BassKernelResults of the most recent run


def _build_nc():
    import concourse.tile as tile
    from concourse import bacc, mybir

    f32 = mybir.dt.float32
    Act = mybir.ActivationFunctionType
    AX = mybir.AxisListType.X

    nc = bacc.Bacc("TRN2", target_bir_lowering=False, debug=False,
                   num_devices=NCORES)
    feat = nc.dram_tensor("feat", [CORE_FLOATS], f32,
                          kind="ExternalInput").ap()
    up = nc.dram_tensor("up", [BL, 128, 512], f32, kind="ExternalInput").ap()
    # outv columns: VEC_TILES totals then p1/pred_add partials per batch;
    # outa columns: ACT_TILES totals
    outv = nc.dram_tensor("outv", [128, len(VEC_TILES) + 2 * BL], f32,
                          kind="ExternalOutput").ap()
    outa = nc.dram_tensor("outa", [128, len(ACT_TILES)], f32,
                          kind="ExternalOutput").ap()

    with tile.TileContext(nc) as tc:
        with (
            tc.tile_pool(name="big", bufs=6) as big,
            tc.tile_pool(name="small", bufs=2) as small,
            tc.tile_pool(name="acc", bufs=1) as accp,
        ):
            obufv = accp.tile([128, len(VEC_TILES) + 2 * BL], f32)
            obufa = accp.tile([128, len(ACT_TILES)], f32)
            dummyv = accp.tile([128, 1], f32)
            dummya = accp.tile([128, 1], f32)

            # pred side: image b as (128, 512), partition p = rows 2p, 2p+1
            for b in range(BL):
                u = small.tile([128, 512], f32)
                nc.gpsimd.dma_start(out=u[:], in_=up[b])
                p1 = small.tile([128, 512], f32)
                nc.scalar.activation(p1[:], u[:], Act.Sigmoid)
                s2 = small.tile([128, 512], f32)
                nc.scalar.activation(s2[:], p1[:], Act.Sigmoid)
                sm = small.tile([128, 512], f32)  # 2 - sigmoid(p1)
                nc.scalar.activation(sm[:], s2[:], Act.Copy, bias=2.0,
                                     scale=-1.0)
                pa = small.tile([128, 512], f32)  # pred_add
                nc.vector.tensor_mul(pa[:], p1[:], sm[:])
                c = len(VEC_TILES) + 2 * b
                nc.vector.reduce_sum(obufv[:, c:c + 1], p1[:], axis=AX)
                nc.vector.reduce_sum(obufv[:, c + 1:c + 2], pa[:], axis=AX)

            # feat side: per-partition totals, one column per tile, with the
            # reduce alternating between VectorE (tensor_scalar accum) and
            # ScalarE (activation Copy accum); the full-size main output is
            # squashed into a zero-stride dummy AP
            vcol = {t: i for i, t in enumerate(VEC_TILES)}
            acol = {t: i for i, t in enumerate(ACT_TILES)}
            for t in range(TILES):
                f = TILE_FREE[t]
                src = feat[128 * TILE_OFS[t]:128 * (TILE_OFS[t] + f)]
                x = big.tile([128, f], f32, tag="x")
                nc.sync.dma_start(out=x[:],
                                  in_=src.rearrange("(p f) -> p f", p=128))
                if t in acol:
                    nc.scalar.activation(
                        dummya.broadcast_to((128, f)), x[:], Act.Copy,
                        accum_out=obufa[:, acol[t]:acol[t] + 1])
                else:
                    c = vcol[t]
                    nc.vector.tensor_scalar(
                        out=dummyv.broadcast_to((128, f)), in0=x[:],
                        scalar1=0.0, scalar2=None,
                        op0=mybir.AluOpType.add,
                        op1=mybir.AluOpType.add,
                        accum_out=obufv[:, c:c + 1])

            nc.scalar.dma_start(out=outv[:], in_=obufv[:])
            nc.scalar.dma_start(out=outa[:], in_=obufa[:])

    nc.compile()
    return nc


def _upsample2(x):
    """Exact bilinear x2, half-pixel centers (align_corners=False), separable.

    x: (..., n) -> (..., 2n) along the last axis.
    out[2i] = 0.25*x[i-1] + 0.75*x[i]; out[2i+1] = 0.75*x[i] + 0.25*x[i+1]
    with edge clamping.
    """
    left = np.concatenate([x[..., :1], x[..., :-1]], axis=-1)
    right = np.concatenate([x[..., 1:], x[..., -1:]], axis=-1)
    even = 0.25 * left + 0.75 * x
    odd = 0.75 * x + 0.25 * right
    out = np.stack([even, odd], axis=-1)
    return out.reshape(*x.shape[:-1], 2 * x.shape[-1])


def _sigmoid(x):
    return 1.0 / (1.0 + np.exp(-x))


def _pred_add(u):
    """pred_add = p1 * (1 - sigmoid(p1)) + p1 for p1 = sigmoid(u)."""
    p1 = _sigmoid(u)
    return p1 * (2.0 - _sigmoid(p1))


def _ln(x, g, b):
    m = x.mean(-1, keepdims=True)
    v = ((x - m) ** 2).mean(-1, keepdims=True)
    return (x - m) / np.sqrt(v + LN_EPS) * g + b


def _conv3x3_sum(W3, bias, S, r_first, r_last, c_first, c_last, x00, x0w,
                 xh0, xhw):
    """Spatial sum of 3x3 zero-pad-1 cross-correlation over a 256x256 image,
    given total S, first/last row sums, first/last col sums, and corners."""
    re = [r_last, 0.0, r_first]   # excluded row sum for tap i = 0,1,2
    ce = [c_last, 0.0, c_first]
    corner = {(0, 0): xhw, (0, 2): xh0, (2, 0): x0w, (2, 2): x00}
    tot = 0.0
    for i in range(3):
        for j in range(3):
            g = S - re[i] - ce[j] + corner.get((i, j), 0.0)
            tot += W3[i, j] * g
    return tot + HW * bias


def _conv1d_sum(W11, bias, S, first5, last5):
    """Spatial sum of a 1x11 (or 11x1) zero-pad-5 cross-correlation given the
    total S and the per-line sums of the first/last 5 lines."""
    tot = 0.0
    for j in range(11):
        if j < 5:
            e = last5[j:].sum()
        elif j > 5:
            e = first5[:j - 5].sum()
        else:
            e = 0.0
        tot += W11[j] * (S - e)
    return tot + HW * bias


def kernel(**inputs):
    from concourse.bass_utils import run_bass_kernel_spmd

    feat = np.ascontiguousarray(np.asarray(inputs["feat"], dtype=np.float32))
    head = np.asarray(inputs["head"], dtype=np.float32)
    pred = np.asarray(inputs["pred"], dtype=np.float32)

    # host: exact bilinear x2 upsample of pred (16,1,128,128) -> (16,256,256)
    up = pred.reshape(BS, 128, 128)
    up = _upsample2(np.swapaxes(_upsample2(np.swapaxes(up, 1, 2)), 1, 2))
    up = np.ascontiguousarray(up, dtype=np.float32)

    if "nc" not in _NC_CACHE:
        _NC_CACHE["nc"] = _build_nc()
    nc = _NC_CACHE["nc"]

    in_maps = []
    for k in range(NCORES):
        in_maps.append({
            "feat": feat[BL * k:BL * (k + 1)].reshape(CORE_FLOATS),
            "up": up[BL * k:BL * (k + 1)].reshape(BL, 128, 512),
        })
    res = run_bass_kernel_spmd(nc, in_maps, list(range(NCORES)), trace=TRACE)
    global LAST_RESULTS
    LAST_RESULTS = res

    # decode: out[p, t] is the sum of a contiguous slice of one image
    img_of = np.empty((TILES, 128), dtype=np.int64)
    for t in range(TILES):
        ps = np.arange(128)
        img_of[t] = (128 * TILE_OFS[t] + ps * TILE_FREE[t]) // HW
    S_all = np.empty((BS, CH), dtype=np.float64)   # per-image totals
    S1 = np.empty((BS,), dtype=np.float64)         # sum of p1 per batch
    S_pa = np.empty((BS,), dtype=np.float64)       # sum of pred_add per batch
    nv = len(VEC_TILES)
    for k in range(NCORES):
        ov = res.results[k]["outv"].astype(np.float64)
        oa = res.results[k]["outa"].astype(np.float64)
        cols = np.empty((TILES, 128), dtype=np.float64)
        cols[VEC_TILES] = ov[:, :nv].T
        cols[ACT_TILES] = oa.T
        s_img = np.zeros(IMGS, dtype=np.float64)
        np.add.at(s_img, img_of.ravel(), cols.ravel())
        S_all[BL * k:BL * (k + 1)] = s_img.reshape(BL, CH)
        for b in range(BL):
            S1[BL * k + b] = ov[:, nv + 2 * b].sum()
            S_pa[BL * k + b] = ov[:, nv + 2 * b + 1].sum()

    f64 = np.float64
    dw_w = np.asarray(inputs["dw_w"], f64)[0, 0]        # (3,3)
    dw_b = float(np.asarray(inputs["dw_b"], f64)[0])
    inc_hw_w = np.asarray(inputs["inc_hw_w"], f64)      # (8,1,3,3)
    inc_hw_b = np.asarray(inputs["inc_hw_b"], f64)
    inc_w_w = np.asarray(inputs["inc_w_w"], f64)        # (8,1,1,11)
    inc_w_b = np.asarray(inputs["inc_w_b"], f64)
    inc_h_w = np.asarray(inputs["inc_h_w"], f64)        # (8,1,11,1)
    inc_h_b = np.asarray(inputs["inc_h_b"], f64)

    fd = feat.astype(f64)
    # border sums for the conv channels (thin slices of feat)
    hw_r0 = fd[:, 40:48, 0, :].sum(-1)        # (16,8) first row sums
    hw_rh = fd[:, 40:48, 255, :].sum(-1)
    hw_c0 = fd[:, 40:48, :, 0].sum(-1)
    hw_ch = fd[:, 40:48, :, 255].sum(-1)
    w_c5 = fd[:, 48:56, :, 0:5].sum(2)        # (16,8,5) first-5 col sums
    w_ce = fd[:, 48:56, :, 251:256].sum(2)
    h_r5 = fd[:, 56:64, 0:5, :].sum(3)        # (16,8,5) first-5 row sums
    h_re = fd[:, 56:64, 251:256, :].sum(3)

    # S_feat[b, c]: spatial sums of feat after the Inception depthwise convs
    S_feat = np.array(S_all)
    for b in range(BS):
        for g in range(8):
            X = fd[b, 40 + g]
            S_feat[b, 40 + g] = _conv3x3_sum(
                inc_hw_w[g, 0], inc_hw_b[g], S_all[b, 40 + g],
                hw_r0[b, g], hw_rh[b, g], hw_c0[b, g], hw_ch[b, g],
                X[0, 0], X[0, 255], X[255, 0], X[255, 255])
            S_feat[b, 48 + g] = _conv1d_sum(
                inc_w_w[g, 0, 0], inc_w_b[g], S_all[b, 48 + g],
                w_c5[b, g], w_ce[b, g])
            S_feat[b, 56 + g] = _conv1d_sum(
                inc_h_w[g, 0, :, 0], inc_h_b[g], S_all[b, 56 + g],
                h_r5[b, g], h_re[b, g])

    # S_pred[b]: spatial sum of p1 + conv3x3(pred_add) + dw_b
    upd = up.astype(f64)
    S_pred = np.empty((BS,), dtype=f64)
    for b in range(BS):
        row0 = _pred_add(upd[b, 0, :])
        rowh = _pred_add(upd[b, 255, :])
        col0 = _pred_add(upd[b, :, 0])
        colh = _pred_add(upd[b, :, 255])
        S_pred[b] = S1[b] + _conv3x3_sum(
            dw_w, dw_b, S_pa[b],
            row0.sum(), rowh.sum(), col0.sum(), colh.sum(),
            row0[0], row0[255], rowh[0], rowh[255])

    # assemble + tiny gated MLP head (exact mirror of the reference)
    assemble = S_pred[:, None] * S_feat                 # (16, 64)
    headd = np.asarray(head, f64).reshape(BS, 1, CH)    # kk = 1

    lin = lambda x, w, b: x @ np.asarray(w, f64).T + np.asarray(b, f64)
    g = lambda n: np.asarray(inputs[n], f64)

    pred_feat = lin(assemble, inputs["pt_w"], inputs["pt_b"])     # (16,128)
    pf_in, pf_out = pred_feat[:, :CH], pred_feat[:, -CH:]
    head_feat = lin(headd, inputs["ht_w"], inputs["ht_b"])        # (16,1,128)
    hf_in, hf_out = head_feat[..., :CH], head_feat[..., -CH:]
    gate = hf_in * pf_in[:, None, :]
    head_gate = _sigmoid(_ln(lin(gate, inputs["hg_w"], inputs["hg_b"]),
                             g("hni_g"), g("hni_b")))
    pred_gate = _sigmoid(_ln(lin(gate, inputs["pg_w"], inputs["pg_b"]),
                             g("pni_g"), g("pni_b")))
    hf_out = _ln(hf_out, g("hno_g"), g("hno_b"))
    pf_out = _ln(pf_out, g("pno_g"), g("pno_b"))
    upd_h = pred_gate * pf_out[:, None, :] + head_gate * hf_out
    upd_h = lin(upd_h, inputs["fc_w"], inputs["fc_b"])
    upd_h = np.maximum(_ln(upd_h, g("fcn_g"), g("fcn_b")), 0.0)   # (16,1,64)
    out = upd_h.reshape(BS, 1, 1, CH).transpose(0, 1, 3, 2)
    return np.ascontiguousarray(out.reshape(BS, 1, CH, 1, 1), dtype=np.float32)



# revision 4
# speedup vs baseline: 1.6724x; 1.6724x over previous
"""HeadUpdator kernel for 8 Trainium2 NeuronCores.

Math: the FFT "assembly" step reduces exactly to
    assemble[b, n, c] = sum_spatial(pred_final[b, n]) * sum_spatial(feat_final[b, c])
because irfft2(rfft2(p) * rfft2(f)) is a circular convolution, and summing a
circular convolution over all output positions factors into the product of the
operand sums.

The spatial sum of each zero-padded depthwise conv output factors as
    sum(conv(x, W)) = sum_k W_k * rect_k(x) + H*W*bias
where rect_k is the sum of x over a rectangle missing up to 5 border rows or
cols.  So the device-side work over the 256 MB `feat` tensor is a pure
streaming per-image total-sum (VectorE free-dim reduces, hidden under the HBM
DMA stream); border corrections are computed on host from thin slices of feat
(10 rows + 10 cols + 4 corners per conv channel).

Device (per core, data-parallel over batch: 2 samples/core):
  - 16 x 2MB feat tiles -> one (128, 4096) -> (128, 1) reduce each.
  - pred: host-upsampled image -> Sigmoid chain on ScalarE -> reduces.
Host: exact bilinear x2 upsample, border/corner corrections, the tiny gated
MLP head (16x64 matmuls), and output assembly.
"""

import numpy as np

BS, CH, H, W = 16, 64, 256, 256
NCORES = 8
BL = BS // NCORES            # local batches per core
IMGS = BL * CH               # images per core
HW = H * W
CORE_FLOATS = IMGS * HW      # 8388608 feat elements per core
# feat streams as fp16 (quantization error on the per-image sums is ~1e-4
# of the cross-channel spread; tolerance is 2e-2), halving HBM traffic.
# Per-tile free-dim sizes (x128 partitions); tapered tail shortens the
# final DMA-dependent reduce.
TILE_FREE = [8192] * 7 + [2048] * 4
TILE_OFS = np.cumsum([0] + TILE_FREE[:-1]).tolist()
TILES = len(TILE_FREE)
assert sum(TILE_FREE) * 128 == CORE_FLOATS
# reduce-engine split: ScalarE (activation accum, 1.2G elem/s/lane) and
# VectorE (tensor_reduce) alternate so the per-tile reduce keeps pace with
# the fp16 DMA stream on both engines
ACT_TILES = [t for t in range(TILES) if t % 2 == 1]
VEC_TILES = [t for t in range(TILES) if t % 2 == 0]
LN_EPS = 1e-5

_NC_CACHE = {}
TRACE = False          # test harness sets True to collect an NTFF profile
LAST_RESULTS = None    # BassKernelResults of the most recent run


def _build_nc():
    import concourse.tile as tile
    from concourse import bacc, mybir

    f32 = mybir.dt.float32
    Act = mybir.ActivationFunctionType
    AX = mybir.AxisListType.X

    f16 = mybir.dt.float16

    nc = bacc.Bacc("TRN2", target_bir_lowering=False, debug=False,
                   num_devices=NCORES)
    feat = nc.dram_tensor("feat", [CORE_FLOATS], f16,
                          kind="ExternalInput").ap()
    up = nc.dram_tensor("up", [BL, 128, 512], f32, kind="ExternalInput").ap()
    # outv columns: VEC_TILES totals then p1/pred_add partials per batch;
    # outa columns: ACT_TILES totals
    outv = nc.dram_tensor("outv", [128, len(VEC_TILES) + 2 * BL], f32,
                          kind="ExternalOutput").ap()
    outa = nc.dram_tensor("outa", [128, len(ACT_TILES)], f32,
                          kind="ExternalOutput").ap()

    with tile.TileContext(nc) as tc:
        with (
            tc.tile_pool(name="big", bufs=6) as big,
            tc.tile_pool(name="small", bufs=2) as small,
            tc.tile_pool(name="acc", bufs=1) as accp,
        ):
            obufv = accp.tile([128, len(VEC_TILES) + 2 * BL], f32)
            obufa = accp.tile([128, len(ACT_TILES)], f32)
            dummya = accp.tile([128, 1], f32)

            # pred side: image b as (128, 512), partition p = rows 2p, 2p+1
            for b in range(BL):
                u = small.tile([128, 512], f32)
                nc.gpsimd.dma_start(out=u[:], in_=up[b])
                p1 = small.tile([128, 512], f32)
                nc.scalar.activation(p1[:], u[:], Act.Sigmoid)
                s2 = small.tile([128, 512], f32)
                nc.scalar.activation(s2[:], p1[:], Act.Sigmoid)
                sm = small.tile([128, 512], f32)  # 2 - sigmoid(p1)
                nc.scalar.activation(sm[:], s2[:], Act.Copy, bias=2.0,
                                     scale=-1.0)
                pa = small.tile([128, 512], f32)  # pred_add
                nc.vector.tensor_mul(pa[:], p1[:], sm[:])
                c = len(VEC_TILES) + 2 * b
                nc.vector.reduce_sum(obufv[:, c:c + 1], p1[:], axis=AX)
                nc.vector.reduce_sum(obufv[:, c + 1:c + 2], pa[:], axis=AX)

            # feat side: per-partition totals, one column per tile, with the
            # reduce alternating between VectorE (tensor_reduce) and ScalarE
            # (activation Copy accum, squashed into a zero-stride dummy AP)
            vcol = {t: i for i, t in enumerate(VEC_TILES)}
            acol = {t: i for i, t in enumerate(ACT_TILES)}
            for t in range(TILES):
                f = TILE_FREE[t]
                src = feat[128 * TILE_OFS[t]:128 * (TILE_OFS[t] + f)]
                x = big.tile([128, f], f16, tag="x")
                nc.sync.dma_start(out=x[:],
                                  in_=src.rearrange("(p f) -> p f", p=128))
                if t in acol:
                    nc.scalar.activation(
                        dummya.broadcast_to((128, f)), x[:], Act.Copy,
                        accum_out=obufa[:, acol[t]:acol[t] + 1])
                else:
                    c = vcol[t]
                    nc.vector.tensor_reduce(
                        out=obufv[:, c:c + 1], in_=x[:], axis=AX,
                        op=mybir.AluOpType.add)

            # two output DMAs on distinct queues so the tails overlap
            nc.scalar.dma_start(out=outa[:], in_=obufa[:])
            nc.sync.dma_start(out=outv[:], in_=obufv[:])

    nc.compile()
    return nc


def _upsample2(x):
    """Exact bilinear x2, half-pixel centers (align_corners=False), separable.

    x: (..., n) -> (..., 2n) along the last axis.
    out[2i] = 0.25*x[i-1] + 0.75*x[i]; out[2i+1] = 0.75*x[i] + 0.25*x[i+1]
    with edge clamping.
    """
    left = np.concatenate([x[..., :1], x[..., :-1]], axis=-1)
    right = np.concatenate([x[..., 1:], x[..., -1:]], axis=-1)
    even = 0.25 * left + 0.75 * x
    odd = 0.75 * x + 0.25 * right
    out = np.stack([even, odd], axis=-1)
    return out.reshape(*x.shape[:-1], 2 * x.shape[-1])


def _sigmoid(x):
    return 1.0 / (1.0 + np.exp(-x))


def _pred_add(u):
    """pred_add = p1 * (1 - sigmoid(p1)) + p1 for p1 = sigmoid(u)."""
    p1 = _sigmoid(u)
    return p1 * (2.0 - _sigmoid(p1))


def _ln(x, g, b):
    m = x.mean(-1, keepdims=True)
    v = ((x - m) ** 2).mean(-1, keepdims=True)
    return (x - m) / np.sqrt(v + LN_EPS) * g + b


def _conv3x3_sum(W3, bias, S, r_first, r_last, c_first, c_last, x00, x0w,
                 xh0, xhw):
    """Spatial sum of 3x3 zero-pad-1 cross-correlation over a 256x256 image,
    given total S, first/last row sums, first/last col sums, and corners."""
    re = [r_last, 0.0, r_first]   # excluded row sum for tap i = 0,1,2
    ce = [c_last, 0.0, c_first]
    corner = {(0, 0): xhw, (0, 2): xh0, (2, 0): x0w, (2, 2): x00}
    tot = 0.0
    for i in range(3):
        for j in range(3):
            g = S - re[i] - ce[j] + corner.get((i, j), 0.0)
            tot += W3[i, j] * g
    return tot + HW * bias


def _conv1d_sum(W11, bias, S, first5, last5):
    """Spatial sum of a 1x11 (or 11x1) zero-pad-5 cross-correlation given the
    total S and the per-line sums of the first/last 5 lines."""
    tot = 0.0
    for j in range(11):
        if j < 5:
            e = last5[j:].sum()
        elif j > 5:
            e = first5[:j - 5].sum()
        else:
            e = 0.0
        tot += W11[j] * (S - e)
    return tot + HW * bias


def kernel(**inputs):
    from concourse.bass_utils import run_bass_kernel_spmd

    feat = np.ascontiguousarray(np.asarray(inputs["feat"], dtype=np.float32))
    head = np.asarray(inputs["head"], dtype=np.float32)
    pred = np.asarray(inputs["pred"], dtype=np.float32)

    # host: exact bilinear x2 upsample of pred (16,1,128,128) -> (16,256,256)
    up = pred.reshape(BS, 128, 128)
    up = _upsample2(np.swapaxes(_upsample2(np.swapaxes(up, 1, 2)), 1, 2))
    up = np.ascontiguousarray(up, dtype=np.float32)

    if "nc" not in _NC_CACHE:
        _NC_CACHE["nc"] = _build_nc()
    nc = _NC_CACHE["nc"]

    feat16 = feat.astype(np.float16)
    in_maps = []
    for k in range(NCORES):
        in_maps.append({
            "feat": feat16[BL * k:BL * (k + 1)].reshape(CORE_FLOATS),
            "up": up[BL * k:BL * (k + 1)].reshape(BL, 128, 512),
        })
    res = run_bass_kernel_spmd(nc, in_maps, list(range(NCORES)), trace=TRACE)
    global LAST_RESULTS
    LAST_RESULTS = res

    # decode: out[p, t] is the sum of a contiguous slice of one image
    img_of = np.empty((TILES, 128), dtype=np.int64)
    for t in range(TILES):
        ps = np.arange(128)
        img_of[t] = (128 * TILE_OFS[t] + ps * TILE_FREE[t]) // HW
    S_all = np.empty((BS, CH), dtype=np.float64)   # per-image totals
    S1 = np.empty((BS,), dtype=np.float64)         # sum of p1 per batch
    S_pa = np.empty((BS,), dtype=np.float64)       # sum of pred_add per batch
    nv = len(VEC_TILES)
    for k in range(NCORES):
        ov = res.results[k]["outv"].astype(np.float64)
        oa = res.results[k]["outa"].astype(np.float64)
        cols = np.empty((TILES, 128), dtype=np.float64)
        cols[VEC_TILES] = ov[:, :nv].T
        cols[ACT_TILES] = oa.T
        s_img = np.zeros(IMGS, dtype=np.float64)
        np.add.at(s_img, img_of.ravel(), cols.ravel())
        S_all[BL * k:BL * (k + 1)] = s_img.reshape(BL, CH)
        for b in range(BL):
            S1[BL * k + b] = ov[:, nv + 2 * b].sum()
            S_pa[BL * k + b] = ov[:, nv + 2 * b + 1].sum()

    f64 = np.float64
    dw_w = np.asarray(inputs["dw_w"], f64)[0, 0]        # (3,3)
    dw_b = float(np.asarray(inputs["dw_b"], f64)[0])
    inc_hw_w = np.asarray(inputs["inc_hw_w"], f64)      # (8,1,3,3)
    inc_hw_b = np.asarray(inputs["inc_hw_b"], f64)
    inc_w_w = np.asarray(inputs["inc_w_w"], f64)        # (8,1,1,11)
    inc_w_b = np.asarray(inputs["inc_w_b"], f64)
    inc_h_w = np.asarray(inputs["inc_h_w"], f64)        # (8,1,11,1)
    inc_h_b = np.asarray(inputs["inc_h_b"], f64)

    fd = feat.astype(f64)
    # border sums for the conv channels (thin slices of feat)
    hw_r0 = fd[:, 40:48, 0, :].sum(-1)        # (16,8) first row sums
    hw_rh = fd[:, 40:48, 255, :].sum(-1)
    hw_c0 = fd[:, 40:48, :, 0].sum(-1)
    hw_ch = fd[:, 40:48, :, 255].sum(-1)
    w_c5 = fd[:, 48:56, :, 0:5].sum(2)        # (16,8,5) first-5 col sums
    w_ce = fd[:, 48:56, :, 251:256].sum(2)
    h_r5 = fd[:, 56:64, 0:5, :].sum(3)        # (16,8,5) first-5 row sums
    h_re = fd[:, 56:64, 251:256, :].sum(3)

    # S_feat[b, c]: spatial sums of feat after the Inception depthwise convs
    S_feat = np.array(S_all)
    for b in range(BS):
        for g in range(8):
            X = fd[b, 40 + g]
            S_feat[b, 40 + g] = _conv3x3_sum(
                inc_hw_w[g, 0], inc_hw_b[g], S_all[b, 40 + g],
                hw_r0[b, g], hw_rh[b, g], hw_c0[b, g], hw_ch[b, g],
                X[0, 0], X[0, 255], X[255, 0], X[255, 255])
            S_feat[b, 48 + g] = _conv1d_sum(
                inc_w_w[g, 0, 0], inc_w_b[g], S_all[b, 48 + g],
                w_c5[b, g], w_ce[b, g])
            S_feat[b, 56 + g] = _conv1d_sum(
                inc_h_w[g, 0, :, 0], inc_h_b[g], S_all[b, 56 + g],
                h_r5[b, g], h_re[b, g])

    # S_pred[b]: spatial sum of p1 + conv3x3(pred_add) + dw_b
    upd = up.astype(f64)
    S_pred = np.empty((BS,), dtype=f64)
    for b in range(BS):
        row0 = _pred_add(upd[b, 0, :])
        rowh = _pred_add(upd[b, 255, :])
        col0 = _pred_add(upd[b, :, 0])
        colh = _pred_add(upd[b, :, 255])
        S_pred[b] = S1[b] + _conv3x3_sum(
            dw_w, dw_b, S_pa[b],
            row0.sum(), rowh.sum(), col0.sum(), colh.sum(),
            row0[0], row0[255], rowh[0], rowh[255])

    # assemble + tiny gated MLP head (exact mirror of the reference)
    assemble = S_pred[:, None] * S_feat                 # (16, 64)
    headd = np.asarray(head, f64).reshape(BS, 1, CH)    # kk = 1

    lin = lambda x, w, b: x @ np.asarray(w, f64).T + np.asarray(b, f64)
    g = lambda n: np.asarray(inputs[n], f64)

    pred_feat = lin(assemble, inputs["pt_w"], inputs["pt_b"])     # (16,128)
    pf_in, pf_out = pred_feat[:, :CH], pred_feat[:, -CH:]
    head_feat = lin(headd, inputs["ht_w"], inputs["ht_b"])        # (16,1,128)
    hf_in, hf_out = head_feat[..., :CH], head_feat[..., -CH:]
    gate = hf_in * pf_in[:, None, :]
    head_gate = _sigmoid(_ln(lin(gate, inputs["hg_w"], inputs["hg_b"]),
                             g("hni_g"), g("hni_b")))
    pred_gate = _sigmoid(_ln(lin(gate, inputs["pg_w"], inputs["pg_b"]),
                             g("pni_g"), g("pni_b")))
    hf_out = _ln(hf_out, g("hno_g"), g("hno_b"))
    pf_out = _ln(pf_out, g("pno_g"), g("pno_b"))
    upd_h = pred_gate * pf_out[:, None, :] + head_gate * hf_out
    upd_h = lin(upd_h, inputs["fc_w"], inputs["fc_b"])
    upd_h = np.maximum(_ln(upd_h, g("fcn_g"), g("fcn_b")), 0.0)   # (16,1,64)
    out = upd_h.reshape(BS, 1, 1, CH).transpose(0, 1, 3, 2)
    return np.ascontiguousarray(out.reshape(BS, 1, CH, 1, 1), dtype=np.float32)



# revision 8
# speedup vs baseline: 1.8854x; 1.1274x over previous
"""HeadUpdator kernel for 8 Trainium2 NeuronCores.

Math: the FFT "assembly" step reduces exactly to
    assemble[b, n, c] = sum_spatial(pred_final[b, n]) * sum_spatial(feat_final[b, c])
because irfft2(rfft2(p) * rfft2(f)) is a circular convolution, and summing a
circular convolution over all output positions factors into the product of the
operand sums.

The spatial sum of each zero-padded depthwise conv output factors as
    sum(conv(x, W)) = sum_k W_k * rect_k(x) + H*W*bias
where rect_k is the sum of x over a rectangle missing up to 5 border rows or
cols.  So the device-side work over the 256 MB `feat` tensor is a pure
streaming per-image total-sum (VectorE free-dim reduces, hidden under the HBM
DMA stream); border corrections are computed on host from thin slices of feat
(10 rows + 10 cols + 4 corners per conv channel).

Device (per core, data-parallel over batch: 2 samples/core):
  - 16 x 2MB feat tiles -> one (128, 4096) -> (128, 1) reduce each.
  - pred: host-upsampled image -> Sigmoid chain on ScalarE -> reduces.
Host: exact bilinear x2 upsample, border/corner corrections, the tiny gated
MLP head (16x64 matmuls), and output assembly.
"""

import numpy as np

BS, CH, H, W = 16, 64, 256, 256
NCORES = 8
BL = BS // NCORES            # local batches per core
IMGS = BL * CH               # images per core
HW = H * W
CORE_FLOATS = IMGS * HW      # 8388608 feat elements per core
# feat streams as fp16 (quantization error on the per-image sums is ~1e-4
# of the cross-channel spread; tolerance is 2e-2), halving HBM traffic.
# Per-tile free-dim sizes (x128 partitions); tapered tail shortens the
# final DMA-dependent reduce.  Engine split matches the 1.25:1 rate ratio
# of ScalarE (1.2 GHz activation accum) to VectorE (0.96 GHz tensor_reduce)
# so both finish together just after the DMA stream ends.
TILE_FREE = [8192] * 7 + [2048] * 3 + [1024] * 2
TILE_OFS = np.cumsum([0] + TILE_FREE[:-1]).tolist()
TILES = len(TILE_FREE)
assert sum(TILE_FREE) * 128 == CORE_FLOATS
ACT_TILES = [0, 2, 4, 6, 8, 11]   # 4*8192 + 2048 + 1024 = 35840 units
VEC_TILES = [1, 3, 5, 7, 9, 10]   # 3*8192 + 2*2048 + 1024 = 29696 units
assert sorted(ACT_TILES + VEC_TILES) == list(range(TILES))
LN_EPS = 1e-5

_NC_CACHE = {}
TRACE = False          # test harness sets True to collect an NTFF profile
LAST_RESULTS = None    # BassKernelResults of the most recent run


def _build_nc():
    import concourse.tile as tile
    from concourse import bacc, mybir

    f32 = mybir.dt.float32
    Act = mybir.ActivationFunctionType
    AX = mybir.AxisListType.X

    f16 = mybir.dt.float16

    nc = bacc.Bacc("TRN2", target_bir_lowering=False, debug=False,
                   num_devices=NCORES)
    feat = nc.dram_tensor("feat", [CORE_FLOATS], f16,
                          kind="ExternalInput").ap()
    up = nc.dram_tensor("up", [BL, 128, 512], f32, kind="ExternalInput").ap()
    # outv columns: VEC_TILES totals then p1/pred_add partials per batch;
    # outa columns: ACT_TILES totals
    outv = nc.dram_tensor("outv", [128, len(VEC_TILES) + 2 * BL], f32,
                          kind="ExternalOutput").ap()
    outa = nc.dram_tensor("outa", [128, len(ACT_TILES)], f32,
                          kind="ExternalOutput").ap()

    with tile.TileContext(nc) as tc:
        with (
            tc.tile_pool(name="big", bufs=7) as big,
            tc.tile_pool(name="tail", bufs=5) as tailp,
            tc.tile_pool(name="small", bufs=4) as small,
            tc.tile_pool(name="acc", bufs=1) as accp,
        ):
            obufv = accp.tile([128, len(VEC_TILES) + 2 * BL], f32)
            obufa = accp.tile([128, len(ACT_TILES)], f32)
            dummya = accp.tile([128, 1], f32)

            # pred side: image b as (128, 512), partition p = rows 2p, 2p+1
            for b in range(BL):
                u = small.tile([128, 512], f32)
                nc.scalar.dma_start(out=u[:], in_=up[b])
                p1 = small.tile([128, 512], f32)
                nc.scalar.activation(p1[:], u[:], Act.Sigmoid)
                s2 = small.tile([128, 512], f32)
                nc.scalar.activation(s2[:], p1[:], Act.Sigmoid)
                sm = small.tile([128, 512], f32)  # 2 - sigmoid(p1)
                nc.scalar.activation(sm[:], s2[:], Act.Copy, bias=2.0,
                                     scale=-1.0)
                pa = small.tile([128, 512], f32)  # pred_add
                nc.vector.tensor_mul(pa[:], p1[:], sm[:])
                c = len(VEC_TILES) + 2 * b
                nc.vector.reduce_sum(obufv[:, c:c + 1], p1[:], axis=AX)
                nc.vector.reduce_sum(obufv[:, c + 1:c + 2], pa[:], axis=AX)

            # feat side: per-partition totals, one column per tile, with the
            # reduce alternating between VectorE (tensor_reduce) and ScalarE
            # (activation Copy accum, squashed into a zero-stride dummy AP)
            vcol = {t: i for i, t in enumerate(VEC_TILES)}
            acol = {t: i for i, t in enumerate(ACT_TILES)}
            for t in range(TILES):
                f = TILE_FREE[t]
                src = feat[128 * TILE_OFS[t]:128 * (TILE_OFS[t] + f)]
                pool = big if f == 8192 else tailp
                x = pool.tile([128, f], f16, tag="x")
                nc.sync.dma_start(out=x[:],
                                  in_=src.rearrange("(p f) -> p f", p=128))
                if t in acol:
                    nc.scalar.activation(
                        dummya.broadcast_to((128, f)), x[:], Act.Copy,
                        accum_out=obufa[:, acol[t]:acol[t] + 1])
                else:
                    c = vcol[t]
                    nc.vector.tensor_reduce(
                        out=obufv[:, c:c + 1], in_=x[:], axis=AX,
                        op=mybir.AluOpType.add)

            # two output DMAs on distinct queues so the tails overlap
            nc.scalar.dma_start(out=outa[:], in_=obufa[:])
            nc.sync.dma_start(out=outv[:], in_=obufv[:])

    nc.compile()
    return nc


def _upsample2(x):
    """Exact bilinear x2, half-pixel centers (align_corners=False), separable.

    x: (..., n) -> (..., 2n) along the last axis.
    out[2i] = 0.25*x[i-1] + 0.75*x[i]; out[2i+1] = 0.75*x[i] + 0.25*x[i+1]
    with edge clamping.
    """
    left = np.concatenate([x[..., :1], x[..., :-1]], axis=-1)
    right = np.concatenate([x[..., 1:], x[..., -1:]], axis=-1)
    even = 0.25 * left + 0.75 * x
    odd = 0.75 * x + 0.25 * right
    out = np.stack([even, odd], axis=-1)
    return out.reshape(*x.shape[:-1], 2 * x.shape[-1])


def _sigmoid(x):
    return 1.0 / (1.0 + np.exp(-x))


def _pred_add(u):
    """pred_add = p1 * (1 - sigmoid(p1)) + p1 for p1 = sigmoid(u)."""
    p1 = _sigmoid(u)
    return p1 * (2.0 - _sigmoid(p1))


def _ln(x, g, b):
    m = x.mean(-1, keepdims=True)
    v = ((x - m) ** 2).mean(-1, keepdims=True)
    return (x - m) / np.sqrt(v + LN_EPS) * g + b


def _conv3x3_sum(W3, bias, S, r_first, r_last, c_first, c_last, x00, x0w,
                 xh0, xhw):
    """Spatial sum of 3x3 zero-pad-1 cross-correlation over a 256x256 image,
    given total S, first/last row sums, first/last col sums, and corners."""
    re = [r_last, 0.0, r_first]   # excluded row sum for tap i = 0,1,2
    ce = [c_last, 0.0, c_first]
    corner = {(0, 0): xhw, (0, 2): xh0, (2, 0): x0w, (2, 2): x00}
    tot = 0.0
    for i in range(3):
        for j in range(3):
            g = S - re[i] - ce[j] + corner.get((i, j), 0.0)
            tot += W3[i, j] * g
    return tot + HW * bias


def _conv1d_sum(W11, bias, S, first5, last5):
    """Spatial sum of a 1x11 (or 11x1) zero-pad-5 cross-correlation given the
    total S and the per-line sums of the first/last 5 lines."""
    tot = 0.0
    for j in range(11):
        if j < 5:
            e = last5[j:].sum()
        elif j > 5:
            e = first5[:j - 5].sum()
        else:
            e = 0.0
        tot += W11[j] * (S - e)
    return tot + HW * bias


def kernel(**inputs):
    from concourse.bass_utils import run_bass_kernel_spmd

    feat = np.ascontiguousarray(np.asarray(inputs["feat"], dtype=np.float32))
    head = np.asarray(inputs["head"], dtype=np.float32)
    pred = np.asarray(inputs["pred"], dtype=np.float32)

    # host: exact bilinear x2 upsample of pred (16,1,128,128) -> (16,256,256)
    up = pred.reshape(BS, 128, 128)
    up = _upsample2(np.swapaxes(_upsample2(np.swapaxes(up, 1, 2)), 1, 2))
    up = np.ascontiguousarray(up, dtype=np.float32)

    if "nc" not in _NC_CACHE:
        _NC_CACHE["nc"] = _build_nc()
    nc = _NC_CACHE["nc"]

    feat16 = feat.astype(np.float16)
    in_maps = []
    for k in range(NCORES):
        in_maps.append({
            "feat": feat16[BL * k:BL * (k + 1)].reshape(CORE_FLOATS),
            "up": up[BL * k:BL * (k + 1)].reshape(BL, 128, 512),
        })
    res = run_bass_kernel_spmd(nc, in_maps, list(range(NCORES)), trace=TRACE)
    global LAST_RESULTS
    LAST_RESULTS = res

    # decode: out[p, t] is the sum of a contiguous slice of one image
    img_of = np.empty((TILES, 128), dtype=np.int64)
    for t in range(TILES):
        ps = np.arange(128)
        img_of[t] = (128 * TILE_OFS[t] + ps * TILE_FREE[t]) // HW
    S_all = np.empty((BS, CH), dtype=np.float64)   # per-image totals
    S1 = np.empty((BS,), dtype=np.float64)         # sum of p1 per batch
    S_pa = np.empty((BS,), dtype=np.float64)       # sum of pred_add per batch
    nv = len(VEC_TILES)
    for k in range(NCORES):
        ov = res.results[k]["outv"].astype(np.float64)
        oa = res.results[k]["outa"].astype(np.float64)
        cols = np.empty((TILES, 128), dtype=np.float64)
        cols[VEC_TILES] = ov[:, :nv].T
        cols[ACT_TILES] = oa.T
        s_img = np.zeros(IMGS, dtype=np.float64)
        np.add.at(s_img, img_of.ravel(), cols.ravel())
        S_all[BL * k:BL * (k + 1)] = s_img.reshape(BL, CH)
        for b in range(BL):
            S1[BL * k + b] = ov[:, nv + 2 * b].sum()
            S_pa[BL * k + b] = ov[:, nv + 2 * b + 1].sum()

    f64 = np.float64
    dw_w = np.asarray(inputs["dw_w"], f64)[0, 0]        # (3,3)
    dw_b = float(np.asarray(inputs["dw_b"], f64)[0])
    inc_hw_w = np.asarray(inputs["inc_hw_w"], f64)      # (8,1,3,3)
    inc_hw_b = np.asarray(inputs["inc_hw_b"], f64)
    inc_w_w = np.asarray(inputs["inc_w_w"], f64)        # (8,1,1,11)
    inc_w_b = np.asarray(inputs["inc_w_b"], f64)
    inc_h_w = np.asarray(inputs["inc_h_w"], f64)        # (8,1,11,1)
    inc_h_b = np.asarray(inputs["inc_h_b"], f64)

    fd = feat.astype(f64)
    # border sums for the conv channels (thin slices of feat)
    hw_r0 = fd[:, 40:48, 0, :].sum(-1)        # (16,8) first row sums
    hw_rh = fd[:, 40:48, 255, :].sum(-1)
    hw_c0 = fd[:, 40:48, :, 0].sum(-1)
    hw_ch = fd[:, 40:48, :, 255].sum(-1)
    w_c5 = fd[:, 48:56, :, 0:5].sum(2)        # (16,8,5) first-5 col sums
    w_ce = fd[:, 48:56, :, 251:256].sum(2)
    h_r5 = fd[:, 56:64, 0:5, :].sum(3)        # (16,8,5) first-5 row sums
    h_re = fd[:, 56:64, 251:256, :].sum(3)

    # S_feat[b, c]: spatial sums of feat after the Inception depthwise convs
    S_feat = np.array(S_all)
    for b in range(BS):
        for g in range(8):
            X = fd[b, 40 + g]
            S_feat[b, 40 + g] = _conv3x3_sum(
                inc_hw_w[g, 0], inc_hw_b[g], S_all[b, 40 + g],
                hw_r0[b, g], hw_rh[b, g], hw_c0[b, g], hw_ch[b, g],
                X[0, 0], X[0, 255], X[255, 0], X[255, 255])
            S_feat[b, 48 + g] = _conv1d_sum(
                inc_w_w[g, 0, 0], inc_w_b[g], S_all[b, 48 + g],
                w_c5[b, g], w_ce[b, g])
            S_feat[b, 56 + g] = _conv1d_sum(
                inc_h_w[g, 0, :, 0], inc_h_b[g], S_all[b, 56 + g],
                h_r5[b, g], h_re[b, g])

    # S_pred[b]: spatial sum of p1 + conv3x3(pred_add) + dw_b
    upd = up.astype(f64)
    S_pred = np.empty((BS,), dtype=f64)
    for b in range(BS):
        row0 = _pred_add(upd[b, 0, :])
        rowh = _pred_add(upd[b, 255, :])
        col0 = _pred_add(upd[b, :, 0])
        colh = _pred_add(upd[b, :, 255])
        S_pred[b] = S1[b] + _conv3x3_sum(
            dw_w, dw_b, S_pa[b],
            row0.sum(), rowh.sum(), col0.sum(), colh.sum(),
            row0[0], row0[255], rowh[0], rowh[255])

    # assemble + tiny gated MLP head (exact mirror of the reference)
    assemble = S_pred[:, None] * S_feat                 # (16, 64)
    headd = np.asarray(head, f64).reshape(BS, 1, CH)    # kk = 1

    lin = lambda x, w, b: x @ np.asarray(w, f64).T + np.asarray(b, f64)
    g = lambda n: np.asarray(inputs[n], f64)

    pred_feat = lin(assemble, inputs["pt_w"], inputs["pt_b"])     # (16,128)
    pf_in, pf_out = pred_feat[:, :CH], pred_feat[:, -CH:]
    head_feat = lin(headd, inputs["ht_w"], inputs["ht_b"])        # (16,1,128)
    hf_in, hf_out = head_feat[..., :CH], head_feat[..., -CH:]
    gate = hf_in * pf_in[:, None, :]
    head_gate = _sigmoid(_ln(lin(gate, inputs["hg_w"], inputs["hg_b"]),
                             g("hni_g"), g("hni_b")))
    pred_gate = _sigmoid(_ln(lin(gate, inputs["pg_w"], inputs["pg_b"]),
                             g("pni_g"), g("pni_b")))
    hf_out = _ln(hf_out, g("hno_g"), g("hno_b"))
    pf_out = _ln(pf_out, g("pno_g"), g("pno_b"))
    upd_h = pred_gate * pf_out[:, None, :] + head_gate * hf_out
    upd_h = lin(upd_h, inputs["fc_w"], inputs["fc_b"])
    upd_h = np.maximum(_ln(upd_h, g("fcn_g"), g("fcn_b")), 0.0)   # (16,1,64)
    out = upd_h.reshape(BS, 1, 1, CH).transpose(0, 1, 3, 2)
    return np.ascontiguousarray(out.reshape(BS, 1, CH, 1, 1), dtype=np.float32)

